# revision 28
# baseline (speedup 1.0000x reference)
"""Causal self-attention (B=2, T=2048, C=1024, H=16, D=64) on 8 trn2 NeuronCores.

Sharding: core i handles batch b = i//4 and heads [4*(i%4), 4*(i%4)+4).
Each core computes QKV projection for its head subset, causal attention, and
its partial output projection. Host sums the 4 per-batch partials (disjoint
head subsets -> the "all-reduce after proj" is a host-side sum) and adds bias.

Engine budget per core (the attention phase is ScalarE-bound):
  - ScalarE runs ONLY the softmax exp ACTIVATEs (~82us) + phase-A QK drains.
  - VectorE: all PSUM drains (V, O, proj), reciprocals, normalize multiplies.
  - GpSimd: causal mask via affine_select on the exp'd P tiles (SBUF), plus
    the h1 cross-partition SWDGE copy.
  - TensorE: QK/V/proj matmuls, attention S/PV (row-tiled head pairs), and
    the reciprocal partition-broadcast as a K=1 matmul into PSUM.
  - Diagonal S/exp/PV tiles are trimmed to n_off = 128*j (bf16 full rate at
    any N).
  - QK phase interleaves all 4 m-blocks across 8 PSUM banks so PE keeps pace
    with the input DMA front (no HAM re-throttle).
  - y is written bf16, one aggregated DMA per 512-row block; host sums in f64.
"""

import numpy as np
import ml_dtypes
from contextlib import ExitStack

B, T, C, H, D = 2, 2048, 1024, 16, 64
NCORES = 8
HEADS_PER_CORE = 4  # 2 head-pairs
CCHUNKS = C // 128  # 8
TBLOCKS = T // 128  # 16
QBLOCKS = T // 512  # 4

_CACHE = {}


def _build():
    import concourse.mybir as mybir
    import concourse.tile as tile
    from concourse import bacc

    F32 = mybir.dt.float32
    BF16 = mybir.dt.bfloat16
    EXPF = mybir.ActivationFunctionType.Exp

    nc = bacc.Bacc("TRN2", target_bir_lowering=False, debug=False,
                   num_devices=NCORES)

    xT = nc.dram_tensor("xT", (C, T), BF16, kind="ExternalInput")
    wqk = nc.dram_tensor("wqk", (C, 512), BF16, kind="ExternalInput")
    wv = nc.dram_tensor("wv", (C, 256), BF16, kind="ExternalInput")
    wp = nc.dram_tensor("wp", (256, C), BF16, kind="ExternalInput")
    y = nc.dram_tensor("y", (T, C), BF16, kind="ExternalOutput")

    with ExitStack() as ctx:
        tc = ctx.enter_context(tile.TileContext(nc))
        const = ctx.enter_context(tc.tile_pool(name="const", bufs=1))
        xw = ctx.enter_context(tc.tile_pool(name="xw", bufs=1))
        qkv = ctx.enter_context(tc.tile_pool(name="qkv", bufs=1))
        ppool = ctx.enter_context(tc.tile_pool(name="ppool", bufs=6))
        misc = ctx.enter_context(tc.tile_pool(name="misc", bufs=2))
        # PSUM budget (8 banks): psMM 4 (QKV/V/proj/O accumulators + r2
        # broadcast, disjoint in time) + psS 2*2 (S double buffer; also
        # lends 4 banks to the QK phase for m-blocks 1,3)
        psMM = ctx.enter_context(tc.tile_pool(name="psMM", bufs=4, space="PSUM"))
        psS = ctx.enter_context(tc.tile_pool(name="psS", bufs=2, space="PSUM"))

        # constants built on-device before the DMA front lands
        ones1 = const.tile([1, D], BF16, name="ones1", tag="ones1")
        nc.vector.memset(ones1, 1.0)
        warm_src = const.tile([128, 640], BF16, name="wsrc", tag="wsrc")
        nc.vector.memset(warm_src, 0.25)
        scrap = const.tile([1, D], BF16, name="scrap", tag="scrap")
        # pre-load the exp ACT table set during the DMA front
        nc.scalar.activation(out=scrap, in_=ones1, func=EXPF)

        # persistent QKV activation tiles
        qT = [qkv.tile([128, T], BF16, name=f"qT{i}", tag=f"qT{i}") for i in range(2)]
        kT = [qkv.tile([128, T], BF16, name=f"kT{i}", tag=f"kT{i}") for i in range(2)]
        vaug = [qkv.tile([128, HEADS_PER_CORE, D + 1], BF16, name=f"va{t}", tag=f"va{t}")
                for t in range(TBLOCKS)]
        for t in range(TBLOCKS):
            nc.vector.memset(vaug[t][:, :, D], 1.0)

        # PE warmup: dummy matmuls keep the HAM activity monitor busy through
        # the DMA front so real matmuls start at 2.4GHz
        warm = psS.tile([128, 2, 512], F32, name="s", tag="s")
        for i in range(10):
            nc.tensor.matmul(warm[:, 0, :], warm_src[:, 0:128], warm_src[:, 128:640],
                             skip_group_check=True)
        ones32 = const.tile([65, D], F32, name="ones32", tag="ones32")
        nc.vector.memset(ones32, 1.0)

        # ---- input DMAs (x chunks interleaved with the weights that unlock
        # the first QK m-block so PE can start as soon as chunk 0 lands) ----
        wqk_t = [None] * CCHUNKS
        wv_t = [None] * CCHUNKS
        xc = [None] * CCHUNKS
        for c in range(CCHUNKS):
            t_ = xw.tile([128, T], BF16, name=f"x{c}", tag=f"x{c}")
            for hf in range(2):
                # first chunks: halves on different engines' DMA queues so
                # the first matmul's data lands at 2x queue bandwidth
                eng = nc.scalar if (c < 2 and hf == 1) else nc.sync
                eng.dma_start(
                    out=t_[:, hf * 1024:(hf + 1) * 1024],
                    in_=xT[c * 128:(c + 1) * 128, hf * 1024:(hf + 1) * 1024])
            xc[c] = t_
            t_ = xw.tile([128, 512], BF16, name=f"wqk{c}", tag=f"wqk{c}")
            nc.gpsimd.dma_start(out=t_, in_=wqk[c * 128:(c + 1) * 128, :])
            wqk_t[c] = t_
        for c in range(CCHUNKS):
            t_ = xw.tile([128, 256], BF16, name=f"wv{c}", tag=f"wv{c}")
            nc.gpsimd.dma_start(out=t_, in_=wv[c * 128:(c + 1) * 128, :])
            wv_t[c] = t_
        wp_t = []
        for ch in range(2):
            t_ = qkv.tile([128, C], BF16, name=f"wp{ch}", tag=f"wp{ch}")
            nc.gpsimd.dma_start(out=t_, in_=wp[ch * 128:(ch + 1) * 128, :])
            wp_t.append(t_)

        QK_DSTS = {0: qT[0], 1: qT[1], 2: kT[0], 3: kT[1]}

        def qk_ng0():
            """First-half QK m-blocks (T cols 0:1024) interleaved per
            x-chunk: 8 matmuls become ready per arriving chunk (paced with
            the DMA front), using psMM (m-blocks 0,2) + psS (1,3) = 8
            banks. Second half (cols 1024:2048) isn't needed until qb2 and
            runs as background work inside the attention phase."""
            pss = {}
            for mb in (0, 2):
                pss[mb] = [psMM.tile([128, 512], F32, name="mm", tag="mm")
                           for _ in range(2)]
            for mb in (1, 3):
                t_ = psS.tile([128, 2, 512], F32, name="s", tag="s")
                pss[mb] = [t_[:, 0, :], t_[:, 1, :]]
            for c in range(CCHUNKS - 1):
                for mb in range(4):
                    lhs = wqk_t[c][:, mb * 128:(mb + 1) * 128]
                    for k in range(2):
                        nc.tensor.matmul(
                            pss[mb][k], lhs, xc[c][:, k * 512:(k + 1) * 512],
                            start=(c == 0), stop=False)
            # last chunk interleaved with the drain copies; psS-backed
            # m-blocks (1,3) first so attention's S psum frees earliest
            c = CCHUNKS - 1
            for mb in (1, 3, 0, 2):
                lhs = wqk_t[c][:, mb * 128:(mb + 1) * 128]
                for k in range(2):
                    nc.tensor.matmul(
                        pss[mb][k], lhs, xc[c][:, k * 512:(k + 1) * 512],
                        start=False, stop=True)
                    nc.scalar.copy(out=QK_DSTS[mb][:, k * 512:(k + 1) * 512],
                                   in_=pss[mb][k])

        def qk_ng1_item(mb, k, half, ps_box):
            """Half of one second-half QK accumulation group (4 of 8
            chunks): small enough (~0.9us) for the S-lookahead window to
            absorb without stalling the exp stream."""
            n = 2 + k
            if half == 0:
                ps_box.append(psMM.tile([128, 512], F32, name="mm", tag="mm"))
            ps = ps_box[0]
            for c in range(4 * half, 4 * half + 4):
                nc.tensor.matmul(
                    ps, wqk_t[c][:, mb * 128:(mb + 1) * 128],
                    xc[c][:, n * 512:(n + 1) * 512],
                    start=(c == 0), stop=(c == CCHUNKS - 1))
            if half == 1:
                nc.scalar.copy(out=QK_DSTS[mb][:, n * 512:(n + 1) * 512],
                               in_=ps)

        def v_tblock(t):
            """V for tokens [t*128, (t+1)*128) -> vaug[t][:, :, 0:64]"""
            ps = psMM.tile([128, 256], F32, name="mm", tag="mm")
            for c in range(CCHUNKS):
                nc.tensor.matmul(ps, xc[c][:, t * 128:(t + 1) * 128], wv_t[c],
                                 start=(c == 0), stop=(c == CCHUNKS - 1))
            nc.vector.tensor_copy(
                out=vaug[t][:, :, 0:D],
                in_=ps.rearrange("p (h d) -> p h d", h=HEADS_PER_CORE))

        stage1_q = []
        stage2_q = []
        comb_ref = {}
        # Background PE work (V blocks, QK second-half, proj subs), drip-fed
        # one item per kb step so bursts never block the next S matmul in
        # the in-order Tensor queue (which would starve the exp stream).
        # Items carry `req`: the qb whose attention block needs their output
        # (BIG = never a prerequisite); blocks force-drain due items first.
        BIG = 99
        bg = []  # list of (req, closure)

        def bg_pop():
            if bg:
                bg.pop(0)[1]()

        def bg_drain_due(qb):
            due = [fn for (req, fn) in bg if req <= qb]
            bg[:] = [(req, fn) for (req, fn) in bg if req > qb]
            for fn in due:
                fn()

        def attention_block(hp, qb, last=False):
            """One q-block of attention for head-pair hp (heads 2hp, 2hp+1).

            The normalization chain for the previous block is emitted in
            two stages inside this block's key loop (reciprocal chain at
            kb==2, broadcast+normalize at kb==5) so its DMA round-trip
            latency never sits at the head of the in-order PE queue.
            """
            bg_drain_due(qb)
            if hp == 0 and qb < QBLOCKS - 1:
                for t in range(4 * qb + 4, 4 * qb + 8):
                    bg.append((qb + 1, lambda t=t: v_tblock(t)))
                if qb in (1, 2):
                    # second-half QK for cols (1+qb)*512: needed by qb+1
                    k = qb - 1
                    for mb in range(4):
                        ps_box = []
                        for half in range(2):
                            bg.append((qb + 1,
                                       lambda mb=mb, k=k, half=half,
                                       ps_box=ps_box:
                                       qk_ng1_item(mb, k, half, ps_box)))
            oaug = [psMM.tile([D + 1, 512], F32, name="mm", tag="mm")
                    for h in range(2)]
            last_kb = 4 * qb + 3

            def emit_pv(kb, pt, n_off):
                for h in range(2):
                    nc.tensor.matmul(
                        oaug[h][:, n_off:512],
                        vaug[kb][:, 2 * hp + h, :],
                        pt[:, h, n_off:512],
                        start=(kb == 0), stop=(kb == last_kb))

            pv_prev = None
            for kb in range(last_kb + 1):
                if kb == 2 and stage1_q:
                    norm_stage1(*stage1_q.pop(0))
                if kb == 5 and stage2_q:
                    norm_stage2(*stage2_q.pop(0))
                j = kb - 4 * qb  # >= 0 on diagonal band
                diag = j >= 0
                n_off = 128 * j if diag else 0
                w = 512 - n_off
                # both heads' S^T into one 2-bank psum tile (row-tiled
                # concurrent matmuls at array rows 0-63 / 64-127)
                sp = psS.tile([128, 2, 512], F32, name="s", tag="s")
                for h in range(2):
                    nc.tensor.matmul(
                        sp[:, h, n_off:512],
                        kT[hp][64 * h:64 * h + 64, kb * 128:(kb + 1) * 128],
                        qT[hp][64 * h:64 * h + 64, qb * 512 + n_off:(qb + 1) * 512])
                # the previous step's PV sits BEHIND this S in the in-order
                # PE queue, so the mask-select round trip on pt(kb-1) never
                # delays the S matmul that gates the next exp
                if pv_prev is not None:
                    emit_pv(*pv_prev)
                bg_pop()
                pt = ppool.tile([128, 2, 512], BF16, name="p", tag="p")
                nc.scalar.activation(out=pt[:, :, n_off:512],
                                     in_=sp[:, :, n_off:512],
                                     func=EXPF, scale=1.0 / np.sqrt(D))
                if diag:
                    # causal mask: keep pt[p, h, i] iff i - p >= 0
                    # (global: q = qb*512 + n_off + i, k = kb*128 + p,
                    #  q - k = i - p when n_off = 128*j)
                    nc.gpsimd.affine_select(
                        out=pt[:, :, n_off:512], in_=pt[:, :, n_off:512],
                        compare_op=mybir.AluOpType.is_ge, fill=0.0,
                        base=0, channel_multiplier=-1,
                        pattern=[[0, 2], [1, w]])
                pv_prev = (kb, pt, n_off)
            emit_pv(*pv_prev)
            # drain O_aug (features + rowsum row) to SBUF right away (frees
            # both psum banks); h1's raw features also start their hop to
            # the upper partitions of the combined proj-operand tile, and
            # the rowsum rows scatter across partitions for the reciprocal.
            ou0 = misc.tile([D + 1, 512], BF16, name=f"ou{hp}0",
                            tag=f"ou{hp}0", bufs=2)
            ou1 = misc.tile([D + 1, 512], BF16, name=f"ou{hp}1",
                            tag=f"ou{hp}1", bufs=2)
            nc.vector.tensor_copy(out=ou0, in_=oaug[0])
            nc.vector.tensor_copy(out=ou1, in_=oaug[1])
            comb = misc.tile([128, 512], BF16, name=f"cb{hp}",
                             tag=f"cb{hp}", bufs=2)
            nc.sync.dma_start(out=comb[64:128, :], in_=ou1[0:D, :])
            # capture the proj operand pair now: stage2 runs two blocks
            # later, when comb_ref[0] already points at the next q-block
            combs = (comb_ref[0], comb) if hp == 1 else None
            comb_ref[hp] = comb
            rb = misc.tile([128, 2, 4], BF16, name="rb", tag="rb")
            for h, out_ in ((0, ou0), (1, ou1)):
                nc.sync.dma_start(
                    out=rb[0:128, h, :].unsqueeze(1),
                    in_=out_[D:D + 1, :].rearrange("p (a b) -> p a b", a=128))
            stage1_q.append((hp, qb, ou0, comb, rb, combs))

        def norm_stage1(hp, qb, ou0, comb, rb, combs):
            """Reciprocal of the scattered rowsums, gathered back to two
            single-partition rows (one per head)."""
            rbi = misc.tile([128, 2, 4], BF16, name="rbi", tag="rbi")
            with nc.allow_low_precision(
                    reason="bf16 softmax denominators are within rel-err budget"):
                nc.vector.reciprocal(out=rbi, in_=rb)
            rinv = [misc.tile([1, 512], BF16, name=f"ri{h}", tag=f"ri{h}")
                    for h in range(2)]
            for h in range(2):
                nc.sync.dma_start(
                    out=rinv[h].rearrange("p (a b) -> p a b", a=128),
                    in_=rbi[0:128, h, :].unsqueeze(1))
            stage2_q.append((hp, qb, ou0, comb,
                             [rinv[0][:, :], rinv[1][:, :]], combs))

        def norm_stage2(hp, qb, ou0, comb, rinv, combs, ones_t=None,
                        tail=False):
            """Broadcast 1/rowsum across 64 partitions via K=1 PE matmuls
            into PSUM, then normalize both heads into `comb` (h0 ->
            partitions 0:64, h1 in-place at 64:128); queue the output
            projection once both head-pairs of this q-block are normalized."""
            if ones_t is None:
                ones_t = ones1
            r2ps = psMM.tile([128, 512], F32, name="mm", tag="mm")
            for h in range(2):
                nc.tensor.matmul(r2ps[64 * h:64 * h + 64, :],
                                 ones_t, rinv[h])
            nc.vector.tensor_mul(comb[0:64, :], ou0[0:D, :], r2ps[0:64, :])
            nc.vector.tensor_mul(comb[64:128, :], comb[64:128, :],
                                 r2ps[64:128, :])
            if hp == 1:
                if tail:
                    proj_sub(qb, 0, combs)
                    proj_sub(qb, 1, combs)
                    _flush_y(qb, 0, 2)
                    proj_sub(qb, 2, combs)
                    proj_sub(qb, 3, combs)
                    _flush_y(qb, 2, 4)
                else:
                    for sub in range(4):
                        bg.append((BIG, lambda sub=sub, qb=qb, combs=combs: (
                            proj_sub(qb, sub, combs),
                            _flush_y(qb, 0, 4) if sub == 3 else None)))

        def proj_sub(qb, sub, combs):
            ytq = ytq_ref.setdefault(qb, misc.tile(
                [128, 4, 2, 512], BF16, name="ytq", tag="ytq", bufs=2))
            ys = [psMM.tile([128, 512], F32, name="mm", tag="mm")
                  for _ in range(2)]
            for hp in range(2):
                lhs = combs[hp][:, sub * 128:(sub + 1) * 128]
                for half in range(2):
                    nc.tensor.matmul(
                        ys[half], lhs,
                        wp_t[hp][:, half * 512:(half + 1) * 512],
                        start=(hp == 0), stop=(hp == 1))
            for half in range(2):
                nc.vector.tensor_copy(out=ytq[:, sub, half, :],
                                      in_=ys[half])

        def _flush_y(qb, s0, s1):
            ytq = ytq_ref[qb]
            nc.sync.dma_start(
                out=y[qb * 512 + s0 * 128:qb * 512 + s1 * 128, :].rearrange(
                    "(s p) (hf c) -> p s hf c", p=128, hf=2),
                in_=ytq[:, s0:s1, :, :])

        ytq_ref = {}

        # Phase A: first-half QK m-blocks interleaved per chunk so PE paces
        # the x-chunk DMA front; V[0] right after (V[1:4] drip in as
        # background items ahead of the PV steps that consume them).
        qk_ng0()
        for t in range(4):
            v_tblock(t)
        # Phase B: attention blocks with V/QK-second-half/proj drip-fed as
        # background items and two-stage deferred normalization.
        for qb in range(QBLOCKS):
            attention_block(0, qb)
            attention_block(1, qb)
        # Tail: leftover background work, then the last two blocks' norm
        # stages. Dummy matmuls sized to the reciprocal chain's DMA round
        # trips keep the HAM clock warm so the final projection runs at
        # full clock.
        while bg:
            bg_pop()
        if stage2_q:
            norm_stage2(*stage2_q.pop(0))
        if stage1_q:
            norm_stage1(*stage1_q.pop(0))
        warm2 = psS.tile([128, 2, 512], F32, name="s", tag="s")
        for i in range(24):
            nc.tensor.matmul(warm2[:, 0, :], warm_src[:, 0:128],
                             warm_src[:, 128:640], skip_group_check=True)
        norm_stage2(*stage2_q.pop(0), tail=True)

    nc.compile()
    return nc


def _get_nc():
    if "nc" not in _CACHE:
        _CACHE["nc"] = _build()
    return _CACHE["nc"]


def _make_in_maps(inputs):
    x = np.asarray(inputs["x"], dtype=np.float32)
    Wqkv = np.asarray(inputs["Wqkv"], dtype=np.float32)
    Wproj = np.asarray(inputs["Wproj"], dtype=np.float32)
    in_maps = []
    for i in range(NCORES):
        b = i // 4
        g = i % 4
        f0 = g * 256  # first feature column of this core's 4 heads
        bf16 = ml_dtypes.bfloat16
        in_maps.append({
            "xT": np.ascontiguousarray(x[b].T.astype(bf16)),
            "wqk": np.ascontiguousarray(
                np.concatenate([Wqkv[:, f0:f0 + 256],
                                Wqkv[:, C + f0:C + f0 + 256]], axis=1).astype(bf16)),
            "wv": np.ascontiguousarray(
                Wqkv[:, 2 * C + f0:2 * C + f0 + 256].astype(bf16)),
            "wp": np.ascontiguousarray(Wproj[f0:f0 + 256, :].astype(bf16)),
        })
    return in_maps


def kernel(x, Wqkv, bqkv, Wproj, bproj):
    from concourse.bass_utils import run_bass_kernel_spmd

    bproj = np.asarray(bproj, dtype=np.float32)
    nc = _get_nc()
    in_maps = _make_in_maps({"x": x, "Wqkv": Wqkv, "Wproj": Wproj})

    res = run_bass_kernel_spmd(nc, in_maps, core_ids=list(range(NCORES)))

    out = np.zeros((B, T, C), dtype=np.float64)
    for i in range(NCORES):
        out[i // 4] += res.results[i]["y"].astype(np.float64)
    out += bproj.astype(np.float64)
    return out.astype(np.float32)


# revision 31
# speedup vs baseline: 1.1612x; 1.1612x over previous
"""Causal self-attention (B=2, T=2048, C=1024, H=16, D=64) on 8 trn2 NeuronCores.

Sharding: core i handles batch b = i//4 and heads [4*(i%4), 4*(i%4)+4).
Each core computes QKV projection for its head subset, causal attention, and
its partial output projection. Host sums the 4 per-batch partials (disjoint
head subsets -> the "all-reduce after proj" is a host-side sum) and adds bias.

Engine budget per core (the attention phase is ScalarE-bound):
  - ScalarE runs ONLY the softmax exp ACTIVATEs (~82us) + phase-A QK drains.
  - VectorE: all PSUM drains (V, O, proj), reciprocals, normalize multiplies.
  - GpSimd: causal mask via affine_select on the exp'd P tiles (SBUF), plus
    the h1 cross-partition SWDGE copy.
  - TensorE: QK/V/proj matmuls, attention S/PV (row-tiled head pairs), and
    the reciprocal partition-broadcast as a K=1 matmul into PSUM.
  - Diagonal S/exp/PV tiles are trimmed to n_off = 128*j (bf16 full rate at
    any N).
  - QK phase interleaves all 4 m-blocks across 8 PSUM banks so PE keeps pace
    with the input DMA front (no HAM re-throttle).
  - y is written bf16, one aggregated DMA per 512-row block; host sums in f64.
"""

import numpy as np
import ml_dtypes
from contextlib import ExitStack

B, T, C, H, D = 2, 2048, 1024, 16, 64
NCORES = 8
HEADS_PER_CORE = 4  # 2 head-pairs
CCHUNKS = C // 128  # 8
TBLOCKS = T // 128  # 16
QBLOCKS = T // 512  # 4

_CACHE = {}


def _build():
    import concourse.mybir as mybir
    import concourse.tile as tile
    from concourse import bacc

    F32 = mybir.dt.float32
    BF16 = mybir.dt.bfloat16
    EXPF = mybir.ActivationFunctionType.Exp

    nc = bacc.Bacc("TRN2", target_bir_lowering=False, debug=False,
                   num_devices=NCORES)

    xT = nc.dram_tensor("xT", (C, T), BF16, kind="ExternalInput")
    wqk = nc.dram_tensor("wqk", (C, 512), BF16, kind="ExternalInput")
    wv = nc.dram_tensor("wv", (C, 256), BF16, kind="ExternalInput")
    wp = nc.dram_tensor("wp", (256, C), BF16, kind="ExternalInput")
    y = nc.dram_tensor("y", (T, C), BF16, kind="ExternalOutput")

    with ExitStack() as ctx:
        tc = ctx.enter_context(tile.TileContext(nc))
        const = ctx.enter_context(tc.tile_pool(name="const", bufs=1))
        xw = ctx.enter_context(tc.tile_pool(name="xw", bufs=1))
        qkv = ctx.enter_context(tc.tile_pool(name="qkv", bufs=1))
        ppool = ctx.enter_context(tc.tile_pool(name="ppool", bufs=4))
        misc = ctx.enter_context(tc.tile_pool(name="misc", bufs=2))
        # PSUM budget (8 banks): psMM 4 (QKV/V/proj/O accumulators + r2
        # broadcast, disjoint in time) + psS 2*2 (S double buffer; also
        # lends 4 banks to the QK phase for m-blocks 1,3)
        psMM = ctx.enter_context(tc.tile_pool(name="psMM", bufs=4, space="PSUM"))
        psS = ctx.enter_context(tc.tile_pool(name="psS", bufs=2, space="PSUM"))

        # constants built on-device before the DMA front lands
        ones1 = const.tile([1, D], BF16, name="ones1", tag="ones1")
        nc.vector.memset(ones1, 1.0)
        warm_src = const.tile([128, 640], BF16, name="wsrc", tag="wsrc")
        nc.vector.memset(warm_src, 0.25)
        scrap = const.tile([1, D], BF16, name="scrap", tag="scrap")
        # pre-load the exp ACT table set during the DMA front
        nc.scalar.activation(out=scrap, in_=ones1, func=EXPF)

        # persistent QKV activation tiles
        qT = [qkv.tile([128, T], BF16, name=f"qT{i}", tag=f"qT{i}") for i in range(2)]
        kT = [qkv.tile([128, T], BF16, name=f"kT{i}", tag=f"kT{i}") for i in range(2)]
        vaug = [qkv.tile([128, HEADS_PER_CORE, D + 1], BF16, name=f"va{t}", tag=f"va{t}")
                for t in range(TBLOCKS)]
        for t in range(TBLOCKS):
            nc.vector.memset(vaug[t][:, :, D], 1.0)

        # PE warmup: dummy matmuls keep the HAM activity monitor busy through
        # the DMA front so real matmuls start at 2.4GHz
        warm = psS.tile([128, 2, 512], F32, name="s", tag="s")
        for i in range(12):
            nc.tensor.matmul(warm[:, 0, :], warm_src[:, 0:128], warm_src[:, 128:640],
                             skip_group_check=True)
        ones32 = const.tile([65, D], F32, name="ones32", tag="ones32")
        nc.vector.memset(ones32, 1.0)

        # ---- input DMAs (x chunks interleaved with the weights that unlock
        # the first QK m-block so PE can start as soon as chunk 0 lands) ----
        wqk_t = [None] * CCHUNKS
        wv_t = [None] * CCHUNKS
        xc = [None] * CCHUNKS
        for c in range(CCHUNKS):
            t_ = xw.tile([128, T], BF16, name=f"x{c}", tag=f"x{c}")
            for hf in range(2):
                # first chunks: halves on different engines' DMA queues so
                # the first matmul's data lands at 2x queue bandwidth
                eng = nc.scalar if (c < 2 and hf == 1) else nc.sync
                eng.dma_start(
                    out=t_[:, hf * 1024:(hf + 1) * 1024],
                    in_=xT[c * 128:(c + 1) * 128, hf * 1024:(hf + 1) * 1024])
            xc[c] = t_
            t_ = xw.tile([128, 512], BF16, name=f"wqk{c}", tag=f"wqk{c}")
            nc.gpsimd.dma_start(out=t_, in_=wqk[c * 128:(c + 1) * 128, :])
            wqk_t[c] = t_
        for c in range(CCHUNKS):
            t_ = xw.tile([128, 256], BF16, name=f"wv{c}", tag=f"wv{c}")
            nc.gpsimd.dma_start(out=t_, in_=wv[c * 128:(c + 1) * 128, :])
            wv_t[c] = t_
        wp_t = []
        for ch in range(2):
            t_ = qkv.tile([128, C], BF16, name=f"wp{ch}", tag=f"wp{ch}")
            nc.gpsimd.dma_start(out=t_, in_=wp[ch * 128:(ch + 1) * 128, :])
            wp_t.append(t_)

        QK_DSTS = {0: qT[0], 1: qT[1], 2: kT[0], 3: kT[1]}

        def qk_ng0():
            """First-half QK m-blocks (T cols 0:1024) interleaved per
            x-chunk: 8 matmuls become ready per arriving chunk (paced with
            the DMA front), using psMM (m-blocks 0,2) + psS (1,3) = 8
            banks. Second half (cols 1024:2048) isn't needed until qb2 and
            runs as background work inside the attention phase."""
            pss = {}
            for mb in (0, 2):
                pss[mb] = [psMM.tile([128, 512], F32, name="mm", tag="mm")
                           for _ in range(2)]
            for mb in (1, 3):
                t_ = psS.tile([128, 2, 512], F32, name="s", tag="s")
                pss[mb] = [t_[:, 0, :], t_[:, 1, :]]
            for c in range(CCHUNKS - 1):
                for mb in range(4):
                    lhs = wqk_t[c][:, mb * 128:(mb + 1) * 128]
                    for k in range(2):
                        nc.tensor.matmul(
                            pss[mb][k], lhs, xc[c][:, k * 512:(k + 1) * 512],
                            start=(c == 0), stop=False)
            # last chunk interleaved with the drain copies; psS-backed
            # m-blocks (1,3) first so attention's S psum frees earliest
            c = CCHUNKS - 1
            for mb in (1, 3, 0, 2):
                lhs = wqk_t[c][:, mb * 128:(mb + 1) * 128]
                for k in range(2):
                    nc.tensor.matmul(
                        pss[mb][k], lhs, xc[c][:, k * 512:(k + 1) * 512],
                        start=False, stop=True)
                    nc.scalar.copy(out=QK_DSTS[mb][:, k * 512:(k + 1) * 512],
                                   in_=pss[mb][k])

        def qk_ng1_item(mb, k):
            n = 2 + k
            ps = psMM.tile([128, 512], F32, name="mm", tag="mm")
            for c in range(CCHUNKS):
                nc.tensor.matmul(
                    ps, wqk_t[c][:, mb * 128:(mb + 1) * 128],
                    xc[c][:, n * 512:(n + 1) * 512],
                    start=(c == 0), stop=(c == CCHUNKS - 1))
            nc.scalar.copy(out=QK_DSTS[mb][:, n * 512:(n + 1) * 512], in_=ps)

        def v_tblock(t):
            """V for tokens [t*128, (t+1)*128) -> vaug[t][:, :, 0:64]"""
            ps = psMM.tile([128, 256], F32, name="mm", tag="mm")
            for c in range(CCHUNKS):
                nc.tensor.matmul(ps, xc[c][:, t * 128:(t + 1) * 128], wv_t[c],
                                 start=(c == 0), stop=(c == CCHUNKS - 1))
            nc.vector.tensor_copy(
                out=vaug[t][:, :, 0:D],
                in_=ps.rearrange("p (h d) -> p h d", h=HEADS_PER_CORE))

        stage1_q = []
        stage2_q = []
        comb_ref = {}
        # Background PE work (V blocks, QK second-half, proj subs), drip-fed
        # one item per kb step so bursts never block the next S matmul in
        # the in-order Tensor queue (which would starve the exp stream).
        # Items carry `req`: the qb whose attention block needs their output
        # (BIG = never a prerequisite); blocks force-drain due items first.
        BIG = 99
        bg = []  # list of (req, closure)

        def bg_pop():
            if bg:
                bg.pop(0)[1]()

        def bg_drain_due(qb):
            due = [fn for (req, fn) in bg if req <= qb]
            bg[:] = [(req, fn) for (req, fn) in bg if req > qb]
            for fn in due:
                fn()

        def attention_block(hp, qb, last=False):
            """One q-block of attention for head-pair hp (heads 2hp, 2hp+1).

            The normalization chain for the previous block is emitted in
            two stages inside this block's key loop (reciprocal chain at
            kb==2, broadcast+normalize at kb==5) so its DMA round-trip
            latency never sits at the head of the in-order PE queue.
            """
            bg_drain_due(qb)
            if hp == 0 and qb < QBLOCKS - 1:
                for t in range(4 * qb + 4, 4 * qb + 8):
                    bg.append((qb + 1, lambda t=t: v_tblock(t)))
                if qb == 0:
                    for k in range(2):
                        for mb in range(4):
                            bg.append((2 + k,
                                       lambda mb=mb, k=k: qk_ng1_item(mb, k)))
            oaug = [psMM.tile([D + 1, 512], F32, name="mm", tag="mm")
                    for h in range(2)]
            last_kb = 4 * qb + 3

            def emit_pv(kb, pt, n_off):
                for h in range(2):
                    nc.tensor.matmul(
                        oaug[h][:, n_off:512],
                        vaug[kb][:, 2 * hp + h, :],
                        pt[:, h, n_off:512],
                        start=(kb == 0), stop=(kb == last_kb))

            pv_prev = None
            for kb in range(last_kb + 1):
                if kb == 2 and stage1_q:
                    norm_stage1(*stage1_q.pop(0))
                if kb == 5 and stage2_q:
                    norm_stage2(*stage2_q.pop(0))
                j = kb - 4 * qb  # >= 0 on diagonal band
                diag = j >= 0
                n_off = 128 * j if diag else 0
                w = 512 - n_off
                # both heads' S^T into one 2-bank psum tile (row-tiled
                # concurrent matmuls at array rows 0-63 / 64-127)
                sp = psS.tile([128, 2, 512], F32, name="s", tag="s")
                for h in range(2):
                    nc.tensor.matmul(
                        sp[:, h, n_off:512],
                        kT[hp][64 * h:64 * h + 64, kb * 128:(kb + 1) * 128],
                        qT[hp][64 * h:64 * h + 64, qb * 512 + n_off:(qb + 1) * 512])
                # the previous step's PV sits BEHIND this S in the in-order
                # PE queue, so the mask-select round trip on pt(kb-1) never
                # delays the S matmul that gates the next exp
                if pv_prev is not None:
                    emit_pv(*pv_prev)
                bg_pop()
                pt = ppool.tile([128, 2, 512], BF16, name="p", tag="p")
                nc.scalar.activation(out=pt[:, :, n_off:512],
                                     in_=sp[:, :, n_off:512],
                                     func=EXPF, scale=1.0 / np.sqrt(D))
                if diag:
                    # causal mask: keep pt[p, h, i] iff i - p >= 0
                    # (global: q = qb*512 + n_off + i, k = kb*128 + p,
                    #  q - k = i - p when n_off = 128*j)
                    nc.gpsimd.affine_select(
                        out=pt[:, :, n_off:512], in_=pt[:, :, n_off:512],
                        compare_op=mybir.AluOpType.is_ge, fill=0.0,
                        base=0, channel_multiplier=-1,
                        pattern=[[0, 2], [1, w]])
                pv_prev = (kb, pt, n_off)
            emit_pv(*pv_prev)
            # drain O_aug (features + rowsum row) to SBUF right away (frees
            # both psum banks); h1's raw features also start their hop to
            # the upper partitions of the combined proj-operand tile, and
            # the rowsum rows scatter across partitions for the reciprocal.
            ou0 = misc.tile([D + 1, 512], BF16, name=f"ou{hp}0",
                            tag=f"ou{hp}0", bufs=2)
            ou1 = misc.tile([D + 1, 512], BF16, name=f"ou{hp}1",
                            tag=f"ou{hp}1", bufs=2)
            nc.vector.tensor_copy(out=ou0, in_=oaug[0])
            nc.vector.tensor_copy(out=ou1, in_=oaug[1])
            comb = misc.tile([128, 512], BF16, name=f"cb{hp}",
                             tag=f"cb{hp}", bufs=2)
            nc.sync.dma_start(out=comb[64:128, :], in_=ou1[0:D, :])
            # capture the proj operand pair now: stage2 runs two blocks
            # later, when comb_ref[0] already points at the next q-block
            combs = (comb_ref[0], comb) if hp == 1 else None
            comb_ref[hp] = comb
            rb = misc.tile([128, 2, 4], BF16, name="rb", tag="rb")
            for h, out_ in ((0, ou0), (1, ou1)):
                nc.sync.dma_start(
                    out=rb[0:128, h, :].unsqueeze(1),
                    in_=out_[D:D + 1, :].rearrange("p (a b) -> p a b", a=128))
            stage1_q.append((hp, qb, ou0, comb, rb, combs))

        def norm_stage1(hp, qb, ou0, comb, rb, combs):
            """Reciprocal of the scattered rowsums, gathered back to two
            single-partition rows (one per head)."""
            rbi = misc.tile([128, 2, 4], BF16, name="rbi", tag="rbi")
            with nc.allow_low_precision(
                    reason="bf16 softmax denominators are within rel-err budget"):
                nc.vector.reciprocal(out=rbi, in_=rb)
            rinv = [misc.tile([1, 512], BF16, name=f"ri{h}", tag=f"ri{h}")
                    for h in range(2)]
            for h in range(2):
                nc.sync.dma_start(
                    out=rinv[h].rearrange("p (a b) -> p a b", a=128),
                    in_=rbi[0:128, h, :].unsqueeze(1))
            stage2_q.append((hp, qb, ou0, comb,
                             [rinv[0][:, :], rinv[1][:, :]], combs))

        def norm_stage2(hp, qb, ou0, comb, rinv, combs, ones_t=None,
                        tail=False):
            """Broadcast 1/rowsum across 64 partitions via K=1 PE matmuls
            into PSUM, then normalize both heads into `comb` (h0 ->
            partitions 0:64, h1 in-place at 64:128); queue the output
            projection once both head-pairs of this q-block are normalized."""
            if ones_t is None:
                ones_t = ones1
            r2ps = psMM.tile([128, 512], F32, name="mm", tag="mm")
            for h in range(2):
                nc.tensor.matmul(r2ps[64 * h:64 * h + 64, :],
                                 ones_t, rinv[h])
            nc.vector.tensor_mul(comb[0:64, :], ou0[0:D, :], r2ps[0:64, :])
            nc.vector.tensor_mul(comb[64:128, :], comb[64:128, :],
                                 r2ps[64:128, :])
            if hp == 1:
                if tail:
                    proj_sub(qb, 0, combs)
                    proj_sub(qb, 1, combs)
                    _flush_y(qb, 0, 2)
                    proj_sub(qb, 2, combs)
                    proj_sub(qb, 3, combs)
                    _flush_y(qb, 2, 4)
                else:
                    for sub in range(4):
                        bg.append((BIG, lambda sub=sub, qb=qb, combs=combs: (
                            proj_sub(qb, sub, combs),
                            _flush_y(qb, 0, 4) if sub == 3 else None)))

        def proj_sub(qb, sub, combs):
            ytq = ytq_ref.setdefault(qb, misc.tile(
                [128, 4, 2, 512], BF16, name="ytq", tag="ytq", bufs=2))
            ys = [psMM.tile([128, 512], F32, name="mm", tag="mm")
                  for _ in range(2)]
            for hp in range(2):
                lhs = combs[hp][:, sub * 128:(sub + 1) * 128]
                for half in range(2):
                    nc.tensor.matmul(
                        ys[half], lhs,
                        wp_t[hp][:, half * 512:(half + 1) * 512],
                        start=(hp == 0), stop=(hp == 1))
            for half in range(2):
                nc.vector.tensor_copy(out=ytq[:, sub, half, :],
                                      in_=ys[half])

        def _flush_y(qb, s0, s1):
            ytq = ytq_ref[qb]
            nc.sync.dma_start(
                out=y[qb * 512 + s0 * 128:qb * 512 + s1 * 128, :].rearrange(
                    "(s p) (hf c) -> p s hf c", p=128, hf=2),
                in_=ytq[:, s0:s1, :, :])

        ytq_ref = {}

        # Phase A: first-half QK m-blocks interleaved per chunk so PE paces
        # the x-chunk DMA front; V[0] right after (V[1:4] drip in as
        # background items ahead of the PV steps that consume them).
        qk_ng0()
        for t in range(4):
            v_tblock(t)
        # Phase B: attention blocks with V/QK-second-half/proj drip-fed as
        # background items and two-stage deferred normalization.
        for qb in range(QBLOCKS):
            attention_block(0, qb)
            attention_block(1, qb)
        # Tail: leftover background work, then the last two blocks' norm
        # stages. Dummy matmuls sized to the reciprocal chain's DMA round
        # trips keep the HAM clock warm so the final projection runs at
        # full clock.
        while bg:
            bg_pop()
        if stage2_q:
            norm_stage2(*stage2_q.pop(0))
        if stage1_q:
            norm_stage1(*stage1_q.pop(0))
        warm2 = psS.tile([128, 2, 512], F32, name="s", tag="s")
        for i in range(24):
            nc.tensor.matmul(warm2[:, 0, :], warm_src[:, 0:128],
                             warm_src[:, 128:640], skip_group_check=True)
        norm_stage2(*stage2_q.pop(0), tail=True)

    nc.compile()
    return nc


def _get_nc():
    if "nc" not in _CACHE:
        _CACHE["nc"] = _build()
    return _CACHE["nc"]


def _make_in_maps(inputs):
    x = np.asarray(inputs["x"], dtype=np.float32)
    Wqkv = np.asarray(inputs["Wqkv"], dtype=np.float32)
    Wproj = np.asarray(inputs["Wproj"], dtype=np.float32)
    in_maps = []
    for i in range(NCORES):
        b = i // 4
        g = i % 4
        f0 = g * 256  # first feature column of this core's 4 heads
        bf16 = ml_dtypes.bfloat16
        in_maps.append({
            "xT": np.ascontiguousarray(x[b].T.astype(bf16)),
            "wqk": np.ascontiguousarray(
                np.concatenate([Wqkv[:, f0:f0 + 256],
                                Wqkv[:, C + f0:C + f0 + 256]], axis=1).astype(bf16)),
            "wv": np.ascontiguousarray(
                Wqkv[:, 2 * C + f0:2 * C + f0 + 256].astype(bf16)),
            "wp": np.ascontiguousarray(Wproj[f0:f0 + 256, :].astype(bf16)),
        })
    return in_maps


def kernel(x, Wqkv, bqkv, Wproj, bproj):
    from concourse.bass_utils import run_bass_kernel_spmd

    bproj = np.asarray(bproj, dtype=np.float32)
    nc = _get_nc()
    in_maps = _make_in_maps({"x": x, "Wqkv": Wqkv, "Wproj": Wproj})

    res = run_bass_kernel_spmd(nc, in_maps, core_ids=list(range(NCORES)))

    out = np.zeros((B, T, C), dtype=np.float64)
    for i in range(NCORES):
        out[i // 4] += res.results[i]["y"].astype(np.float64)
    out += bproj.astype(np.float64)
    return out.astype(np.float32)


# revision 33
# speedup vs baseline: 1.1720x; 1.0092x over previous
"""Causal self-attention (B=2, T=2048, C=1024, H=16, D=64) on 8 trn2 NeuronCores.

Sharding: core i handles batch b = i//4 and heads [4*(i%4), 4*(i%4)+4).
Each core computes QKV projection for its head subset, causal attention, and
its partial output projection. Host sums the 4 per-batch partials (disjoint
head subsets -> the "all-reduce after proj" is a host-side sum) and adds bias.

Engine budget per core (the attention phase is ScalarE-bound):
  - ScalarE runs ONLY the softmax exp ACTIVATEs (~82us) + phase-A QK drains.
  - VectorE: all PSUM drains (V, O, proj), reciprocals, normalize multiplies.
  - GpSimd: causal mask via affine_select on the exp'd P tiles (SBUF), plus
    the h1 cross-partition SWDGE copy.
  - TensorE: QK/V/proj matmuls, attention S/PV (row-tiled head pairs), and
    the reciprocal partition-broadcast as a K=1 matmul into PSUM.
  - Diagonal S/exp/PV tiles are trimmed to n_off = 128*j (bf16 full rate at
    any N).
  - QK phase interleaves all 4 m-blocks across 8 PSUM banks so PE keeps pace
    with the input DMA front (no HAM re-throttle).
  - y is written bf16, one aggregated DMA per 512-row block; host sums in f64.
"""

import numpy as np
import ml_dtypes
from contextlib import ExitStack

B, T, C, H, D = 2, 2048, 1024, 16, 64
NCORES = 8
HEADS_PER_CORE = 4  # 2 head-pairs
CCHUNKS = C // 128  # 8
TBLOCKS = T // 128  # 16
QBLOCKS = T // 512  # 4

_CACHE = {}


def _build():
    import concourse.mybir as mybir
    import concourse.tile as tile
    from concourse import bacc

    F32 = mybir.dt.float32
    BF16 = mybir.dt.bfloat16
    EXPF = mybir.ActivationFunctionType.Exp

    nc = bacc.Bacc("TRN2", target_bir_lowering=False, debug=False,
                   num_devices=NCORES)

    xT = nc.dram_tensor("xT", (C, T), BF16, kind="ExternalInput")
    wqk = nc.dram_tensor("wqk", (C, 512), BF16, kind="ExternalInput")
    wv = nc.dram_tensor("wv", (C, 256), BF16, kind="ExternalInput")
    wp = nc.dram_tensor("wp", (256, C), BF16, kind="ExternalInput")
    y = nc.dram_tensor("y", (T, C), BF16, kind="ExternalOutput")

    with ExitStack() as ctx:
        tc = ctx.enter_context(tile.TileContext(nc))
        const = ctx.enter_context(tc.tile_pool(name="const", bufs=1))
        xw = ctx.enter_context(tc.tile_pool(name="xw", bufs=1))
        qkv = ctx.enter_context(tc.tile_pool(name="qkv", bufs=1))
        ppool = ctx.enter_context(tc.tile_pool(name="ppool", bufs=4))
        misc = ctx.enter_context(tc.tile_pool(name="misc", bufs=2))
        # PSUM budget (8 banks): psMM 4 (QKV/V/proj/O accumulators + r2
        # broadcast, disjoint in time) + psS 2*2 (S double buffer; also
        # lends 4 banks to the QK phase for m-blocks 1,3)
        psMM = ctx.enter_context(tc.tile_pool(name="psMM", bufs=4, space="PSUM"))
        psS = ctx.enter_context(tc.tile_pool(name="psS", bufs=2, space="PSUM"))

        # constants built on-device before the DMA front lands
        ones1 = const.tile([1, D], BF16, name="ones1", tag="ones1")
        nc.vector.memset(ones1, 1.0)
        warm_src = const.tile([128, 640], BF16, name="wsrc", tag="wsrc")
        nc.vector.memset(warm_src, 0.25)
        scrap = const.tile([1, D], BF16, name="scrap", tag="scrap")
        # pre-load the exp ACT table set during the DMA front
        nc.scalar.activation(out=scrap, in_=ones1, func=EXPF)

        # persistent QKV activation tiles
        qT = [qkv.tile([128, T], BF16, name=f"qT{i}", tag=f"qT{i}") for i in range(2)]
        kT = [qkv.tile([128, T], BF16, name=f"kT{i}", tag=f"kT{i}") for i in range(2)]
        vaug = [qkv.tile([128, HEADS_PER_CORE, D + 1], BF16, name=f"va{t}", tag=f"va{t}")
                for t in range(TBLOCKS)]
        for t in range(TBLOCKS):
            nc.vector.memset(vaug[t][:, :, D], 1.0)

        # PE warmup: dummy matmuls keep the HAM activity monitor busy through
        # the DMA front so real matmuls start at 2.4GHz
        warm = psS.tile([128, 2, 512], F32, name="s", tag="s")
        for i in range(12):
            nc.tensor.matmul(warm[:, 0, :], warm_src[:, 0:128], warm_src[:, 128:640],
                             skip_group_check=True)
        ones32 = const.tile([65, D], F32, name="ones32", tag="ones32")
        nc.vector.memset(ones32, 1.0)

        # ---- input DMAs (x chunks interleaved with the weights that unlock
        # the first QK m-block so PE can start as soon as chunk 0 lands) ----
        wqk_t = [None] * CCHUNKS
        wv_t = [None] * CCHUNKS
        xc = [None] * CCHUNKS
        for c in range(CCHUNKS):
            t_ = xw.tile([128, T], BF16, name=f"x{c}", tag=f"x{c}")
            for hf in range(2):
                # first chunks: halves on different engines' DMA queues so
                # the first matmul's data lands at 2x queue bandwidth
                eng = nc.scalar if (c < 2 and hf == 1) else nc.sync
                eng.dma_start(
                    out=t_[:, hf * 1024:(hf + 1) * 1024],
                    in_=xT[c * 128:(c + 1) * 128, hf * 1024:(hf + 1) * 1024])
            xc[c] = t_
            t_ = xw.tile([128, 512], BF16, name=f"wqk{c}", tag=f"wqk{c}")
            nc.gpsimd.dma_start(out=t_, in_=wqk[c * 128:(c + 1) * 128, :])
            wqk_t[c] = t_
        for c in range(CCHUNKS):
            t_ = xw.tile([128, 256], BF16, name=f"wv{c}", tag=f"wv{c}")
            nc.gpsimd.dma_start(out=t_, in_=wv[c * 128:(c + 1) * 128, :])
            wv_t[c] = t_
        wp_t = []
        for ch in range(2):
            t_ = qkv.tile([128, C], BF16, name=f"wp{ch}", tag=f"wp{ch}")
            nc.gpsimd.dma_start(out=t_, in_=wp[ch * 128:(ch + 1) * 128, :])
            wp_t.append(t_)

        QK_DSTS = {0: qT[0], 1: qT[1], 2: kT[0], 3: kT[1]}

        def qk_ng0():
            """First-half QK m-blocks (T cols 0:1024) interleaved per
            x-chunk: 8 matmuls become ready per arriving chunk (paced with
            the DMA front), using psMM (m-blocks 0,2) + psS (1,3) = 8
            banks. Second half (cols 1024:2048) isn't needed until qb2 and
            runs as background work inside the attention phase."""
            pss = {}
            for mb in (0, 2):
                pss[mb] = [psMM.tile([128, 512], F32, name="mm", tag="mm")
                           for _ in range(2)]
            for mb in (1, 3):
                t_ = psS.tile([128, 2, 512], F32, name="s", tag="s")
                pss[mb] = [t_[:, 0, :], t_[:, 1, :]]
            for c in range(CCHUNKS):
                for mb in range(4):
                    lhs = wqk_t[c][:, mb * 128:(mb + 1) * 128]
                    for k in range(2):
                        nc.tensor.matmul(
                            pss[mb][k], lhs, xc[c][:, k * 512:(k + 1) * 512],
                            start=(c == 0), stop=(c == CCHUNKS - 1))
            for mb in range(4):
                for k in range(2):
                    nc.scalar.copy(out=QK_DSTS[mb][:, k * 512:(k + 1) * 512],
                                   in_=pss[mb][k])

        def qk_ng1_item(mb, k):
            n = 2 + k
            ps = psMM.tile([128, 512], F32, name="mm", tag="mm")
            for c in range(CCHUNKS):
                nc.tensor.matmul(
                    ps, wqk_t[c][:, mb * 128:(mb + 1) * 128],
                    xc[c][:, n * 512:(n + 1) * 512],
                    start=(c == 0), stop=(c == CCHUNKS - 1))
            nc.scalar.copy(out=QK_DSTS[mb][:, n * 512:(n + 1) * 512], in_=ps)

        def v_tblock(t):
            """V for tokens [t*128, (t+1)*128) -> vaug[t][:, :, 0:64]"""
            ps = psMM.tile([128, 256], F32, name="mm", tag="mm")
            for c in range(CCHUNKS):
                nc.tensor.matmul(ps, xc[c][:, t * 128:(t + 1) * 128], wv_t[c],
                                 start=(c == 0), stop=(c == CCHUNKS - 1))
            nc.vector.tensor_copy(
                out=vaug[t][:, :, 0:D],
                in_=ps.rearrange("p (h d) -> p h d", h=HEADS_PER_CORE))

        stage1_q = []
        stage2_q = []
        comb_ref = {}
        # Background PE work (V blocks, QK second-half, proj subs), drip-fed
        # one item per kb step so bursts never block the next S matmul in
        # the in-order Tensor queue (which would starve the exp stream).
        # Items carry `req`: the qb whose attention block needs their output
        # (BIG = never a prerequisite); blocks force-drain due items first.
        BIG = 99
        bg = []  # list of (req, closure)

        def bg_pop():
            if bg:
                bg.pop(0)[1]()

        def bg_drain_due(qb):
            due = [fn for (req, fn) in bg if req <= qb]
            bg[:] = [(req, fn) for (req, fn) in bg if req > qb]
            for fn in due:
                fn()

        def attention_block(hp, qb, last=False):
            """One q-block of attention for head-pair hp (heads 2hp, 2hp+1).

            The normalization chain for the previous block is emitted in
            two stages inside this block's key loop (reciprocal chain at
            kb==2, broadcast+normalize at kb==5) so its DMA round-trip
            latency never sits at the head of the in-order PE queue.
            """
            bg_drain_due(qb)
            if hp == 0 and qb < QBLOCKS - 1:
                for t in range(4 * qb + 4, 4 * qb + 8):
                    bg.append((qb + 1, lambda t=t: v_tblock(t)))
                if qb == 0:
                    for k in range(2):
                        for mb in range(4):
                            bg.append((2 + k,
                                       lambda mb=mb, k=k: qk_ng1_item(mb, k)))
            oaug = [psMM.tile([D + 1, 512], F32, name="mm", tag="mm")
                    for h in range(2)]
            last_kb = 4 * qb + 3

            def emit_pv(kb, pt, n_off):
                for h in range(2):
                    nc.tensor.matmul(
                        oaug[h][:, n_off:512],
                        vaug[kb][:, 2 * hp + h, :],
                        pt[:, h, n_off:512],
                        start=(kb == 0), stop=(kb == last_kb))

            pv_prev = None
            for kb in range(last_kb + 1):
                if kb == 2 and stage1_q:
                    norm_stage1(*stage1_q.pop(0))
                if kb == 5 and stage2_q:
                    norm_stage2(*stage2_q.pop(0))
                j = kb - 4 * qb  # >= 0 on diagonal band
                diag = j >= 0
                n_off = 128 * j if diag else 0
                w = 512 - n_off
                # both heads' S^T into one 2-bank psum tile (row-tiled
                # concurrent matmuls at array rows 0-63 / 64-127)
                sp = psS.tile([128, 2, 512], F32, name="s", tag="s")
                for h in range(2):
                    nc.tensor.matmul(
                        sp[:, h, n_off:512],
                        kT[hp][64 * h:64 * h + 64, kb * 128:(kb + 1) * 128],
                        qT[hp][64 * h:64 * h + 64, qb * 512 + n_off:(qb + 1) * 512])
                # the previous step's PV sits BEHIND this S in the in-order
                # PE queue, so the mask-select round trip on pt(kb-1) never
                # delays the S matmul that gates the next exp
                if pv_prev is not None:
                    emit_pv(*pv_prev)
                bg_pop()
                pt = ppool.tile([128, 2, 512], BF16, name="p", tag="p")
                nc.scalar.activation(out=pt[:, :, n_off:512],
                                     in_=sp[:, :, n_off:512],
                                     func=EXPF, scale=1.0 / np.sqrt(D))
                if diag:
                    # causal mask: keep pt[p, h, i] iff i - p >= 0
                    # (global: q = qb*512 + n_off + i, k = kb*128 + p,
                    #  q - k = i - p when n_off = 128*j)
                    nc.gpsimd.affine_select(
                        out=pt[:, :, n_off:512], in_=pt[:, :, n_off:512],
                        compare_op=mybir.AluOpType.is_ge, fill=0.0,
                        base=0, channel_multiplier=-1,
                        pattern=[[0, 2], [1, w]])
                pv_prev = (kb, pt, n_off)
            emit_pv(*pv_prev)
            # drain O_aug (features + rowsum row) to SBUF right away (frees
            # both psum banks); h1's raw features also start their hop to
            # the upper partitions of the combined proj-operand tile, and
            # the rowsum rows scatter across partitions for the reciprocal.
            ou0 = misc.tile([D + 1, 512], BF16, name=f"ou{hp}0",
                            tag=f"ou{hp}0", bufs=2)
            ou1 = misc.tile([D + 1, 512], BF16, name=f"ou{hp}1",
                            tag=f"ou{hp}1", bufs=2)
            nc.vector.tensor_copy(out=ou0, in_=oaug[0])
            nc.vector.tensor_copy(out=ou1, in_=oaug[1])
            comb = misc.tile([128, 512], BF16, name=f"cb{hp}",
                             tag=f"cb{hp}", bufs=2)
            nc.sync.dma_start(out=comb[64:128, :], in_=ou1[0:D, :])
            # capture the proj operand pair now: stage2 runs two blocks
            # later, when comb_ref[0] already points at the next q-block
            combs = (comb_ref[0], comb) if hp == 1 else None
            comb_ref[hp] = comb
            rb = misc.tile([128, 2, 4], BF16, name="rb", tag="rb")
            for h, out_ in ((0, ou0), (1, ou1)):
                nc.sync.dma_start(
                    out=rb[0:128, h, :].unsqueeze(1),
                    in_=out_[D:D + 1, :].rearrange("p (a b) -> p a b", a=128))
            stage1_q.append((hp, qb, ou0, comb, rb, combs))

        def norm_stage1(hp, qb, ou0, comb, rb, combs):
            """Reciprocal of the scattered rowsums, gathered back to two
            single-partition rows (one per head)."""
            rbi = misc.tile([128, 2, 4], BF16, name="rbi", tag="rbi")
            with nc.allow_low_precision(
                    reason="bf16 softmax denominators are within rel-err budget"):
                nc.vector.reciprocal(out=rbi, in_=rb)
            rinv = [misc.tile([1, 512], BF16, name=f"ri{h}", tag=f"ri{h}")
                    for h in range(2)]
            for h in range(2):
                nc.sync.dma_start(
                    out=rinv[h].rearrange("p (a b) -> p a b", a=128),
                    in_=rbi[0:128, h, :].unsqueeze(1))
            stage2_q.append((hp, qb, ou0, comb,
                             [rinv[0][:, :], rinv[1][:, :]], combs))

        def norm_stage2(hp, qb, ou0, comb, rinv, combs, ones_t=None,
                        tail=False):
            """Broadcast 1/rowsum across 64 partitions via K=1 PE matmuls
            into PSUM, then normalize both heads into `comb` (h0 ->
            partitions 0:64, h1 in-place at 64:128); queue the output
            projection once both head-pairs of this q-block are normalized."""
            if ones_t is None:
                ones_t = ones1
            r2ps = psMM.tile([128, 512], F32, name="mm", tag="mm")
            for h in range(2):
                nc.tensor.matmul(r2ps[64 * h:64 * h + 64, :],
                                 ones_t, rinv[h])
            nc.vector.tensor_mul(comb[0:64, :], ou0[0:D, :], r2ps[0:64, :])
            nc.vector.tensor_mul(comb[64:128, :], comb[64:128, :],
                                 r2ps[64:128, :])
            if hp == 1:
                if tail:
                    proj_sub(qb, 0, combs)
                    proj_sub(qb, 1, combs)
                    _flush_y(qb, 0, 2)
                    proj_sub(qb, 2, combs)
                    proj_sub(qb, 3, combs)
                    _flush_y(qb, 2, 4)
                else:
                    for sub in range(4):
                        bg.append((BIG, lambda sub=sub, qb=qb, combs=combs: (
                            proj_sub(qb, sub, combs),
                            _flush_y(qb, 0, 4) if sub == 3 else None)))

        def proj_sub(qb, sub, combs):
            ytq = ytq_ref.setdefault(qb, misc.tile(
                [128, 4, 2, 512], BF16, name="ytq", tag="ytq", bufs=2))
            ys = [psMM.tile([128, 512], F32, name="mm", tag="mm")
                  for _ in range(2)]
            for hp in range(2):
                lhs = combs[hp][:, sub * 128:(sub + 1) * 128]
                for half in range(2):
                    nc.tensor.matmul(
                        ys[half], lhs,
                        wp_t[hp][:, half * 512:(half + 1) * 512],
                        start=(hp == 0), stop=(hp == 1))
            for half in range(2):
                nc.vector.tensor_copy(out=ytq[:, sub, half, :],
                                      in_=ys[half])

        def _flush_y(qb, s0, s1):
            ytq = ytq_ref[qb]
            nc.sync.dma_start(
                out=y[qb * 512 + s0 * 128:qb * 512 + s1 * 128, :].rearrange(
                    "(s p) (hf c) -> p s hf c", p=128, hf=2),
                in_=ytq[:, s0:s1, :, :])

        ytq_ref = {}

        # Phase A: first-half QK m-blocks interleaved per chunk so PE paces
        # the x-chunk DMA front; V[0] right after (V[1:4] drip in as
        # background items ahead of the PV steps that consume them).
        qk_ng0()
        for t in range(4):
            v_tblock(t)
        # Phase B: attention blocks with V/QK-second-half/proj drip-fed as
        # background items and two-stage deferred normalization.
        for qb in range(QBLOCKS):
            attention_block(0, qb)
            attention_block(1, qb)
        # Tail: leftover background work, then the last two blocks' norm
        # stages. Dummy matmuls sized to the reciprocal chain's DMA round
        # trips keep the HAM clock warm so the final projection runs at
        # full clock.
        while bg:
            bg_pop()
        if stage2_q:
            norm_stage2(*stage2_q.pop(0))
        if stage1_q:
            norm_stage1(*stage1_q.pop(0))
        warm2 = psS.tile([128, 2, 512], F32, name="s", tag="s")
        for i in range(10):
            nc.tensor.matmul(warm2[:, 0, 0:384], warm_src[:, 0:128],
                             warm_src[:, 128:512], skip_group_check=True)
        norm_stage2(*stage2_q.pop(0), tail=True)

    nc.compile()
    return nc


def _get_nc():
    if "nc" not in _CACHE:
        _CACHE["nc"] = _build()
    return _CACHE["nc"]


def _make_in_maps(inputs):
    x = np.asarray(inputs["x"], dtype=np.float32)
    Wqkv = np.asarray(inputs["Wqkv"], dtype=np.float32)
    Wproj = np.asarray(inputs["Wproj"], dtype=np.float32)
    in_maps = []
    for i in range(NCORES):
        b = i // 4
        g = i % 4
        f0 = g * 256  # first feature column of this core's 4 heads
        bf16 = ml_dtypes.bfloat16
        in_maps.append({
            "xT": np.ascontiguousarray(x[b].T.astype(bf16)),
            "wqk": np.ascontiguousarray(
                np.concatenate([Wqkv[:, f0:f0 + 256],
                                Wqkv[:, C + f0:C + f0 + 256]], axis=1).astype(bf16)),
            "wv": np.ascontiguousarray(
                Wqkv[:, 2 * C + f0:2 * C + f0 + 256].astype(bf16)),
            "wp": np.ascontiguousarray(Wproj[f0:f0 + 256, :].astype(bf16)),
        })
    return in_maps


def kernel(x, Wqkv, bqkv, Wproj, bproj):
    from concourse.bass_utils import run_bass_kernel_spmd

    bproj = np.asarray(bproj, dtype=np.float32)
    nc = _get_nc()
    in_maps = _make_in_maps({"x": x, "Wqkv": Wqkv, "Wproj": Wproj})

    res = run_bass_kernel_spmd(nc, in_maps, core_ids=list(range(NCORES)))

    out = np.zeros((B, T, C), dtype=np.float64)
    for i in range(NCORES):
        out[i // 4] += res.results[i]["y"].astype(np.float64)
    out += bproj.astype(np.float64)
    return out.astype(np.float32)
